# revision 24
# baseline (speedup 1.0000x reference)
"""Causal linear attention (ELU+1 feature map) on 8 trn2 NeuronCores.

Sharding: core i handles batch b=i//2, sequence half h=i%2 (T=2048 -> 1024
tokens/core).  Second-half cores recompute the first half's running state
S0 = sum_tau phi(k_tau) [v_tau, 1]  (128x129, col 128 = z) from k/v of the
first half; first-half cores get zeroed aux inputs so their S0 == 0.

Math per core (chunk C=128, 8 own chunks + 8 "pre" state-only chunks):
  phi(y) = min(exp(y), 1) + relu(y)            (== ELU(y)+1 exactly)
  y is produced in PSUM with the bias already added (ones-row matmul), so
  phi costs 3 elementwise ops: Exp (ACT), min-with-1 (DVE 4x), and one
  fused scalar_tensor_tensor  (y max 0) add min(e,1)  on DVE or Pool.
  A^T_c = K_c Q_c^T (bf16 PSUM); Am = mask * A fused into the PSUM->SBUF move
  O_c = Am^T.T @ [V_c, 1] + Q_c @ (Se + So)  (den accumulates in col 128)
  out_c = O_c[:, :128] / O_c[:, 128]  (single tensor_scalar divide, bf16 out)
"""

import numpy as np

B, T, D, DV = 4, 2048, 128, 128
H = T // 2          # tokens per core
C = 128             # chunk
NCH = H // C        # chunks per half
NCORES = 8
VW = DV + 1

# bf16 pack columns: [WTb | mask | ident | bias | kTp | qT | kT | vp | v]
OFF_WTB = 0
OFF_MASK = OFF_WTB + D
OFF_ID = OFF_MASK + C
OFF_BIAS = OFF_ID + C
OFF_KTP = OFF_BIAS + 1
OFF_QT = OFF_KTP + H
OFF_KT = OFF_QT + H
OFF_VP = OFF_KT + H
OFF_V = OFF_VP + NCH * VW
B16_COLS = OFF_V + NCH * VW

CFG = {
    # per phi-slice: order [pre1, pre2, k1, q1, k2, q2]
    # chain: exp (ACT, PSUM->SBUF) -> min-with-1 (pool, in-place SBUF) ->
    # fused (y max 0) add e (stt on DVE, frees PSUM).  gpsimd cannot touch
    # PSUM, so masks/scales/snaps split across ACT/DVE only.
    "min_eng": ["pool"] * 6,
    "mask_eng": ["dve"] * 8,
    "div_eng": ["act", "dve", "act", "dve", "act", "dve", "act", "dve"],
    "snap_eng": ["act"] * 9,
    "ktok_eng": ["act", "dve", "act"],   # chunks 0-2 PE-transpose+copy
    "warm_mms": 5,
    "out_pieces": 2,
}

_cache = {}


def _build(cfg=None):
    import concourse.bacc as bacc
    import concourse.tile as tile
    from concourse import mybir
    from bass_rust import add_dep_helper

    cfg = dict(CFG, **(cfg or {}))
    F32 = mybir.dt.float32
    BF16 = mybir.dt.bfloat16
    AF = mybir.ActivationFunctionType
    ALU = mybir.AluOpType

    nc = bacc.Bacc(None, target_bir_lowering=False, debug=False,
                   num_devices=NCORES)

    bin_ = nc.declare_dram_parameter("bin", [D, B16_COLS], BF16, isOutput=False)
    btile = nc.declare_dram_parameter("btile", [1, H], BF16, isOutput=False)
    out = nc.declare_dram_parameter("out", [C, NCH * DV], BF16, isOutput=True)

    def eng_of(which):
        return {"dve": nc.vector, "act": nc.scalar, "pool": nc.gpsimd}[which]

    def copy_eng(which, dst, srcp):
        if which == "act":
            nc.scalar.activation(dst, srcp, AF.Copy)
        else:
            eng_of(which).tensor_copy(dst, srcp)

    with tile.TileContext(nc) as tc:
        with (
            tc.tile_pool(name="cst", bufs=1) as cst,
            tc.tile_pool(name="io", bufs=1) as io,
            tc.tile_pool(name="phi", bufs=1) as phip,
            tc.tile_pool(name="am", bufs=NCH) as amp,
            tc.tile_pool(name="wrk", bufs=2) as wrk,
            tc.tile_pool(name="ps_pre", bufs=3, space="PSUM") as ps_pre,
            tc.tile_pool(name="ps_s", bufs=2, space="PSUM") as ps_s,
            tc.tile_pool(name="ps_a", bufs=1, space="PSUM") as ps_a,
            tc.tile_pool(name="ps_o", bufs=2, space="PSUM") as ps_o,
        ):
            # ---- warm the ACT table while DMAs run ----
            s_warm = cst.tile([D, 1], F32)
            nc.vector.memset(s_warm, 0.0)
            s_warm2 = cst.tile([D, 1], BF16)
            nc.scalar.activation(s_warm2, s_warm, AF.Exp)

            # ---- loads ----
            s_b16 = io.tile([D, B16_COLS], BF16)
            s_btile = cst.tile([1, H], BF16)
            s_ones = cst.tile([1, C], BF16)
            s_ones512 = cst.tile([1, 512], BF16)
            nc.vector.memset(s_ones, 1.0)
            nc.vector.memset(s_ones512, 1.0)

            # ---- PE p-state warmup: keep PE busy so the clock ramps ----
            for _ in range(cfg["warm_mms"]):
                wtile = ps_pre.tile([C, 512], F32, tag="pre")
                nc.tensor.matmul(wtile, s_ones, s_ones512,
                                 start=True, stop=True)

            # need-ordered input pieces; kTp first half via pool SWDGE
            # (parallel DGE path), btile via the DVE HWDGE queue, rest on sync
            hh = H // 2
            nc.gpsimd.dma_start(out=s_b16[:, OFF_KTP:OFF_KTP + hh],
                                in_=bin_[:, OFF_KTP:OFF_KTP + hh])
            nc.scalar.dma_start(out=s_btile, in_=btile[:, :])
            sync_pieces = [
                (0, OFF_KTP),                       # consts (WTb first need)
                (OFF_KTP + hh, OFF_QT),             # kTp 2nd half
                (OFF_KT, OFF_VP),                   # kT
                (OFF_QT, OFF_KT),                   # qT
                (OFF_VP, OFF_V),                    # vp
                (OFF_V, B16_COLS),                  # v
            ]
            for a, b in sync_pieces:
                nc.sync.dma_start(out=s_b16[:, a:b], in_=bin_[:, a:b])

            s_bias = s_b16[:, OFF_BIAS:OFF_BIAS + 1]
            sWTb = s_b16[:, OFF_WTB:OFF_WTB + D]
            s_mask = s_b16[:, OFF_MASK:OFF_MASK + C]
            s_ident = s_b16[:, OFF_ID:OFF_ID + C]
            s_brow = s_btile[:, 0:D]                # b as a [1, D] row

            def vsl(c):
                return s_b16[:, OFF_V + VW * c:OFF_V + VW * (c + 1)]

            def vpsl(c):
                return s_b16[:, OFF_VP + VW * c:OFF_VP + VW * (c + 1)]

            # parity-split state accumulators [D, DV+1]
            NPAR = 2
            Sp = []
            for i in range(NPAR):
                S_i = ps_s.tile([D, DV + 1], F32, tag="s")
                Sp.append(S_i)
            started = [False] * NPAR
            s_first = [None] * NPAR

            # staging for phi
            phi_t = phip.tile([C, H], BF16)       # token-major K_pre
            e_t = phip.tile([C, H], BF16)
            phi_f = phip.tile([D, 2 * H], BF16)   # [Q^T | K^T] feature-major
            e_f = phip.tile([D, 2 * H], BF16)

            slice_idx = [0]

            def phi_ops(pst, e_sl, dst_sl):
                """pst [*,512] PSUM fp32 holds y with bias already added.
                exp -> min (pool, SBUF in-place) -> fused (y max 0) add e."""
                si = slice_idx[0]
                slice_idx[0] += 1
                nc.scalar.activation(e_sl, pst, AF.Exp)
                eng_of(cfg["min_eng"][si]).tensor_scalar(
                    out=e_sl, in0=e_sl, scalar1=1.0, scalar2=None,
                    op0=ALU.min)
                nc.vector.scalar_tensor_tensor(
                    out=dst_sl, in0=pst, scalar=0.0, in1=e_sl,
                    op0=ALU.max, op1=ALU.add)

            # ---- token-major pre for K_pre (state recompute path first) ----
            for j in range(H // 512):
                pst = ps_pre.tile([C, 512], F32, tag="pre")
                prev = nc.tensor.matmul(pst, s_ones,
                                 s_btile[:, 512 * j:512 * (j + 1)],
                                 start=True, stop=False)
                for cc in range(4):
                    c = 4 * j + cc
                    mm_c = nc.tensor.matmul(pst[:, C * cc:C * (cc + 1)],
                                     s_b16[:, OFF_KTP + C * c:OFF_KTP + C * (c + 1)],
                                     sWTb, start=False, stop=(cc == 3))
                    add_dep_helper(mm_c.ins, prev.ins, sync=False,
                                   reason="psum group order")
                    prev = mm_c
                sl = slice(512 * j, 512 * (j + 1))
                phi_ops(pst, e_t[:, sl], phi_t[:, sl])
                # pre-half state contributions (zeros on half-0 cores)
                for cc in range(4):
                    c = 4 * j + cc
                    p = c % NPAR
                    mm_s = nc.tensor.matmul(Sp[p], phi_t[:, C * c:C * (c + 1)],
                                     vpsl(c),
                                     start=(not started[p]), stop=False,
                                     skip_group_check=True)
                    if started[p]:
                        add_dep_helper(mm_s.ins, s_first[p].ins, sync=False,
                                       reason="psum group order")
                    s_first[p] = mm_s
                    started[p] = True

            # ---- feature-major phi for own q, k (bias via ones-row mm) ----
            def phi_slice(i, off, j):
                pre = ps_pre.tile([D, 512], F32, tag="pre")
                mm_w = nc.tensor.matmul(pre, sWTb,
                                 s_b16[:, off + 512 * j:off + 512 * (j + 1)],
                                 start=True, stop=False)
                mm_b = nc.tensor.matmul(pre, s_brow, s_ones512,
                                        start=False, stop=True)
                add_dep_helper(mm_b.ins, mm_w.ins, sync=False,
                               reason="psum group order")
                sl = slice(H * i + 512 * j, H * i + 512 * (j + 1))
                phi_ops(pre, e_f[:, sl], phi_f[:, sl])

            QT = phi_f[:, 0:H]
            KT = phi_f[:, H:2 * H]
            ktok = phip.tile([C, H], BF16)
            Am = [None] * NCH

            def prep_chunks(cs):
                for c in cs:
                    if c < len(cfg["ktok_eng"]):
                        trp = ps_o.tile([C, C], BF16, tag="o")
                        nc.tensor.transpose(trp, KT[:, C * c:C * (c + 1)],
                                            s_ident)
                        copy_eng(cfg["ktok_eng"][c],
                                 ktok[:, C * c:C * (c + 1)], trp)
                for c in cs:
                    A = ps_a.tile([C, C], F32, tag="a")
                    nc.tensor.matmul(A, KT[:, C * c:C * (c + 1)],
                                     QT[:, C * c:C * (c + 1)],
                                     start=True, stop=True)
                    am_c = amp.tile([C, C], BF16, tag="am")
                    Am[c] = am_c
                    eng_of(cfg["mask_eng"][c]).tensor_tensor(
                        out=am_c, in0=A, in1=s_mask, op=ALU.mult)

            outstage = phip.tile([C, NCH * DV], BF16)
            snaps = [None] * NPAR

            def snap_state(p, eng):
                snp = wrk.tile([D, DV + 1], BF16, tag=f"snap{p}")
                copy_eng(eng, snp, Sp[p])
                return snp

            def run_chunk(c):
                p = c % NPAR
                # state update first -- no dependency on O(c)
                mm_su = nc.tensor.matmul(Sp[p], ktok[:, C * c:C * (c + 1)],
                                 vsl(c), start=False,
                                 stop=(c >= NCH - NPAR),
                                 skip_group_check=True)
                add_dep_helper(mm_su.ins, s_first[p].ins, sync=False,
                               reason="psum group order")
                s_first[p] = mm_su
                # snapshot parity p (now through chunk c) for O(c+1); O(c)
                # still reads the previous snaps
                new_snap = None
                if c < NCH - 1:
                    new_snap = snap_state(p, cfg["snap_eng"][c + 1])

                O = ps_o.tile([C, DV + 1], F32, tag="o")
                prev_o = nc.tensor.matmul(O, Am[c], vsl(c), start=True,
                                          stop=False)
                for qi, sn in enumerate(snaps):
                    mm_q = nc.tensor.matmul(O, QT[:, C * c:C * (c + 1)], sn,
                                     start=False, stop=(qi == len(snaps) - 1))
                    add_dep_helper(mm_q.ins, prev_o.ins, sync=False,
                                   reason="psum group order")
                    prev_o = mm_q
                if new_snap is not None:
                    snaps[p] = new_snap

                rec = wrk.tile([C, 1], F32, tag="rec")
                nc.vector.reciprocal(rec, O[:, DV:DV + 1])
                if cfg["div_eng"][c] == "act":
                    nc.scalar.activation(outstage[:, DV * c:DV * (c + 1)],
                                         O[:, 0:DV], AF.Copy, bias=0.0,
                                         scale=rec)
                else:
                    nc.vector.tensor_scalar_mul(
                        outstage[:, DV * c:DV * (c + 1)], O[:, 0:DV], rec)

                np_ = cfg["out_pieces"]
                if np_ == 2:
                    if c in (NCH // 2 - 1, NCH - 1):
                        a = 0 if c == NCH // 2 - 1 else NCH * DV // 2
                        b_ = NCH * DV // 2 if c == NCH // 2 - 1 else NCH * DV
                        nc.sync.dma_start(out=out[:, a:b_],
                                          in_=outstage[:, a:b_])
                else:
                    if c % 2 == 1:
                        nc.sync.dma_start(
                            out=out[:, DV * (c - 1):DV * (c + 1)],
                            in_=outstage[:, DV * (c - 1):DV * (c + 1)])

            phi_slice(1, OFF_KT, 0)
            # ktok for chunk 3 via DMA transpose (chunks 0-2 use PE+copy)
            nc.sync.dma_start_transpose(out=ktok[:, 3 * C:4 * C],
                                        in_=KT[:, 3 * C:4 * C])
            phi_slice(0, OFF_QT, 0)
            # initial snapshots (pre-half state) for O(0)
            for p_ in range(NPAR):
                snaps[p_] = snap_state(p_, cfg["snap_eng"][0])
            prep_chunks(range(0, 4))
            for c in range(4):
                run_chunk(c)
            phi_slice(1, OFF_KT, 1)
            nc.sync.dma_start_transpose(
                out=ktok[:, 4 * C:8 * C].rearrange("t (c d) -> t c d", c=4),
                in_=KT[:, 4 * C:8 * C])
            phi_slice(0, OFF_QT, 1)
            prep_chunks(range(4, NCH))
            for c in range(4, NCH):
                run_chunk(c)

    nc.compile()
    return nc


def _get_nc():
    if "nc" not in _cache:
        _cache["nc"] = _build()
    return _cache["nc"]


def _pack_inputs(q, k, v, W_phi, b_phi):
    import ml_dtypes
    bf16 = ml_dtypes.bfloat16

    WT = np.ascontiguousarray(W_phi.T)                    # [d, e]
    maskm = np.triu(np.ones((C, C), np.float32))          # keep tau <= t
    ident = np.eye(C, dtype=np.float32)
    btile = np.tile(b_phi, NCH).reshape(1, H).astype(bf16)

    def aug(vh):  # [H, DV] -> [C, NCH*(DV+1)] partition-major with ones col
        a = np.concatenate([vh, np.ones((H, 1), np.float32)], axis=1)
        return a.reshape(NCH, C, VW).transpose(1, 0, 2).reshape(C, NCH * VW)

    zeros_vp = np.zeros((C, NCH * VW), np.float32)
    zeros_ktp = np.zeros((D, H), np.float32)

    in_maps = []
    for core in range(NCORES):
        b_idx, half = divmod(core, 2)
        sl = slice(half * H, (half + 1) * H)
        b16 = np.empty((D, B16_COLS), np.float32)
        b16[:, OFF_WTB:OFF_WTB + D] = WT
        b16[:, OFF_MASK:OFF_MASK + C] = maskm
        b16[:, OFF_ID:OFF_ID + C] = ident
        b16[:, OFF_BIAS] = b_phi
        b16[:, OFF_QT:OFF_QT + H] = q[b_idx, sl].T
        b16[:, OFF_KT:OFF_KT + H] = k[b_idx, sl].T
        if half == 1:
            b16[:, OFF_KTP:OFF_KTP + H] = k[b_idx, 0:H].T
            b16[:, OFF_VP:OFF_VP + NCH * VW] = aug(v[b_idx, 0:H])
        else:
            b16[:, OFF_KTP:OFF_KTP + H] = zeros_ktp
            b16[:, OFF_VP:OFF_VP + NCH * VW] = zeros_vp
        b16[:, OFF_V:OFF_V + NCH * VW] = aug(v[b_idx, sl])
        in_maps.append({"bin": b16.astype(bf16), "btile": btile})
    return in_maps


def kernel(q, k, v, W_phi, b_phi):
    from concourse.bass_utils import run_bass_kernel_spmd

    q = np.asarray(q, np.float32)
    k = np.asarray(k, np.float32)
    v = np.asarray(v, np.float32)
    W_phi = np.asarray(W_phi, np.float32)
    b_phi = np.asarray(b_phi, np.float32)

    in_maps = _pack_inputs(q, k, v, W_phi, b_phi)
    nc = _get_nc()
    res = run_bass_kernel_spmd(nc, in_maps, list(range(NCORES)))

    out = np.empty((B, T, DV), np.float32)
    for core in range(NCORES):
        b_idx, half = divmod(core, 2)
        o = np.asarray(res.results[core]["out"],
                       dtype=np.float32)                  # [C, NCH*DV]
        o = o.reshape(C, NCH, DV).transpose(1, 0, 2).reshape(H, DV)
        out[b_idx, half * H:(half + 1) * H] = o
    return out
